# revision 5
# baseline (speedup 1.0000x reference)
"""LMHSA (conv-augmented multi-head self-attention block) on 8 trn2 NeuronCores.

Sharding: data-parallel over batch N=8 -> one batch per core; params + B
replicated. The whole-tensor LayerNorm stats are computed redundantly on every
core from the full x (each core's xf input is the full x rolled so its own
batch is rows 0:256).

Per-core layout convention: features on partitions, spatial (hw=1024) on the
free dim.  Attention is computed in S^T orientation (kv on partitions, q on
free) so softmax-exp output feeds the AV matmul directly with no transposes;
the softmax denominator comes from an appended ones-column in v ("v_aug").
Softmax max-subtraction is skipped (logits are O(1), exp cannot overflow).
"""

import numpy as np

import concourse.bass as bass
import concourse.mybir as mybir
import concourse.tile as tile
from concourse import bacc
from concourse.bass_utils import run_bass_kernel_spmd

N_CORES = 8
C = 256
HW = 1024
HEADS = 8
DK = 32
DV = 32
EPS = 1e-5
P = 128
F32 = mybir.dt.float32
F32R = mybir.dt.float32r
AF = mybir.ActivationFunctionType
ALU = mybir.AluOpType

# matmul input dtype: float32r streams 1 row/cycle (vs 4 for float32) when the
# output free dim is >= 256; storage is identical fp32 bits.
FP32R = True


def _mm(t):
    return t


def build_nc():
    nc = bacc.Bacc("TRN2", target_bir_lowering=False, debug=False,
                   num_devices=N_CORES)

    xf = nc.dram_tensor("xf", [2048, HW], F32, kind="ExternalInput").ap()
    wq = nc.dram_tensor("wq", [C, C], F32R, kind="ExternalInput").ap()
    bq = nc.dram_tensor("bq", [C], F32, kind="ExternalInput").ap()
    wk = nc.dram_tensor("wk", [C, C], F32R, kind="ExternalInput").ap()
    bk = nc.dram_tensor("bk", [C], F32, kind="ExternalInput").ap()
    wv = nc.dram_tensor("wv", [C, C], F32R, kind="ExternalInput").ap()
    bv = nc.dram_tensor("bv", [C], F32, kind="ExternalInput").ap()
    wo = nc.dram_tensor("wo", [C, C], F32R, kind="ExternalInput").ap()
    bo = nc.dram_tensor("bo", [C], F32, kind="ExternalInput").ap()
    ck = nc.dram_tensor("ck", [9, C, C], F32R, kind="ExternalInput").ap()
    cbk = nc.dram_tensor("cbk", [C], F32, kind="ExternalInput").ap()
    cv = nc.dram_tensor("cv", [9, C, C], F32R, kind="ExternalInput").ap()
    cbv = nc.dram_tensor("cbv", [C], F32, kind="ExternalInput").ap()
    bt = nc.dram_tensor("bt", [HEADS, HW, HW], F32, kind="ExternalInput").ap()
    out = nc.dram_tensor("out", [HW, C], F32, kind="ExternalOutput").ap()

    def bcast_pp(ap, n_part):
        # partition-broadcast view of a 1-D dram AP
        return bass.AP(tensor=ap.tensor, offset=ap.offset,
                       ap=[[0, n_part]] + [list(a) for a in ap.ap])

    with tile.TileContext(nc) as tc:
        consts = tc.alloc_tile_pool(name="consts", bufs=1)
        work = tc.alloc_tile_pool(name="work", bufs=1)
        xs_pool = tc.alloc_tile_pool(name="xs", bufs=3)
        bt_pool = tc.alloc_tile_pool(name="btp", bufs=6)
        pt_pool = tc.alloc_tile_pool(name="ptp", bufs=4)
        small = tc.alloc_tile_pool(name="small", bufs=2)

        # ---- constants / weights to SBUF ----
        ck_sb, cv_sb, wq_sb, wk_sb, wv_sb, wo_sb = [], [], [], [], [], []
        for kc in range(2):
            t = consts.tile([P, 9, C], F32R, tag=f"ck{kc}")
            nc.sync.dma_start(out=t, in_=ck[:, kc * P:(kc + 1) * P, :]
                              .rearrange("t k o -> k t o"))
            ck_sb.append(t)
            t = consts.tile([P, 9, C], F32R, tag=f"cv{kc}")
            nc.sync.dma_start(out=t, in_=cv[:, kc * P:(kc + 1) * P, :]
                              .rearrange("t k o -> k t o"))
            cv_sb.append(t)
            for nm, src, lst in (("wq", wq, wq_sb), ("wk", wk, wk_sb),
                                 ("wv", wv, wv_sb), ("wo", wo, wo_sb)):
                t = consts.tile([P, C], F32R, tag=f"{nm}{kc}")
                nc.sync.dma_start(out=t, in_=src[kc * P:(kc + 1) * P, :])
                lst.append(t)

        bq_sb = consts.tile([P, 2], F32, tag="bq")
        nc.sync.dma_start(out=bq_sb, in_=bq.rearrange("(m k) -> k m", k=P))
        bk_sb = consts.tile([P, 2], F32, tag="bk")
        nc.sync.dma_start(out=bk_sb, in_=bk.rearrange("(m k) -> k m", k=P))
        cbk_sb = consts.tile([P, 2], F32, tag="cbk")
        nc.sync.dma_start(out=cbk_sb, in_=cbk.rearrange("(m k) -> k m", k=P))
        cbv_sb = consts.tile([P, 2], F32, tag="cbv")
        nc.sync.dma_start(out=cbv_sb, in_=cbv.rearrange("(m k) -> k m", k=P))
        bv_bc = consts.tile([P, C], F32, tag="bvb")
        nc.sync.dma_start(out=bv_bc, in_=bcast_pp(bv, P))
        bo_bc = consts.tile([P, C], F32, tag="bob")
        nc.sync.dma_start(out=bo_bc, in_=bcast_pp(bo, P))

        eps_t = consts.tile([1, 1], F32, tag="eps")
        nc.vector.memset(eps_t, EPS)
        ones_col = consts.tile([P, 1], F32, tag="ones")
        nc.vector.memset(ones_col, 1.0)

        # ---- padded x for the convs (borders zero) ----
        xpad = []
        for kc in range(2):
            t = work.tile([P, 34, 34], F32R, tag=f"xpad{kc}")
            nc.gpsimd.memset(t.bitcast(F32), 0.0)
            nc.sync.dma_start(
                out=t[:, 1:33, 1:33],
                in_=xf[kc * P:(kc + 1) * P, :].rearrange("c (h w) -> c h w", h=32).bitcast(F32R))
            xpad.append(t)

        # ---- whole-tensor layernorm stats over full x (8 MB) ----
        stats_sb = work.tile([P, 32, 6], F32, tag="stats")
        for t_i in range(16):
            xs = xs_pool.tile([P, HW], F32, tag="xs")
            nc.sync.dma_start(out=xs, in_=xf[t_i * P:(t_i + 1) * P, :])
            for c_i in range(2):
                nc.vector.bn_stats(out=stats_sb[:, t_i * 2 + c_i, :],
                                   in_=xs[:, c_i * 512:(c_i + 1) * 512])
        mv = work.tile([P, 2], F32, tag="mv")
        nc.vector.bn_aggr(out=mv, in_=stats_sb)
        s12 = work.tile([P, 2], F32, tag="s12")
        nc.vector.tensor_mul(s12[:, 1:2], mv[:, 0:1], mv[:, 0:1])
        nc.vector.tensor_add(s12[:, 1:2], s12[:, 1:2], mv[:, 1:2])
        nc.vector.tensor_copy(s12[:, 0:1], mv[:, 0:1])

        with tc.tile_pool(name="ps_st", bufs=1, space="PSUM") as ps_st_pool:
            ps_st = ps_st_pool.tile([1, 2], F32, tag="ps_st")
            nc.tensor.matmul(ps_st, lhsT=ones_col, rhs=s12, start=True,
                             stop=True)
            sg = work.tile([1, 2], F32, tag="sg")
            nc.scalar.mul(sg, ps_st, 1.0 / P)
        var_t = work.tile([1, 1], F32, tag="var")
        nc.vector.tensor_mul(var_t, sg[:, 0:1], sg[:, 0:1])
        nc.vector.tensor_sub(var_t, sg[:, 1:2], var_t)
        sd_t = work.tile([1, 1], F32, tag="sd")
        nc.scalar.activation(sd_t, var_t, AF.Sqrt, bias=eps_t)
        s_t = work.tile([1, 1], F32, tag="st")
        nc.vector.reciprocal(s_t, sd_t)
        t_t = work.tile([1, 1], F32, tag="tt")
        nc.vector.tensor_mul(t_t, sg[:, 0:1], s_t)
        nc.scalar.mul(t_t, t_t, -1.0)
        st_small = work.tile([1, 2], F32, tag="stsm")
        nc.vector.tensor_copy(st_small[:, 0:1], s_t)
        nc.vector.tensor_copy(st_small[:, 1:2], t_t)
        st_bc = work.tile([P, 2], F32, tag="stbc")
        nc.gpsimd.partition_broadcast(st_bc, st_small)
        s_bc, t_bc = st_bc[:, 0:1], st_bc[:, 1:2]

        # ---- xn = (x - m) * rsqrt(var + eps)  (only q uses xn) ----
        xn_sb = []
        for kc in range(2):
            t = work.tile([P, HW], F32R, tag=f"xn{kc}")
            nc.vector.tensor_scalar(t, xpad[kc][:, 1:33, 1:33], s_bc, t_bc,
                                    op0=ALU.mult, op1=ALU.add)
            xn_sb.append(t)

        # ---- convs: yk = conv3x3(x, ck) + cbk ; yv likewise ----
        yk_sb = [work.tile([P, HW], F32R, tag=f"yk{mo}", name=f"yk{mo}") for mo in range(2)]
        yv_sb = [work.tile([P, HW], F32R, tag=f"yv{mo}", name=f"yv{mo}") for mo in range(2)]
        with tc.tile_pool(name="ps_conv", bufs=2, space="PSUM") as ps_conv:
            for cw_sb, cb_sb, y_sb in ((ck_sb, cbk_sb, yk_sb),
                                       (cv_sb, cbv_sb, yv_sb)):
                for mo in range(2):
                    for n in range(2):
                        ps = ps_conv.tile([P, 512], F32, tag="conv")
                        idx = 0
                        for tap in range(9):
                            dy, dx = tap // 3, tap % 3
                            for kc in range(2):
                                nc.tensor.matmul(
                                    ps,
                                    lhsT=_mm(cw_sb[kc][:, tap,
                                                       mo * P:(mo + 1) * P]),
                                    rhs=_mm(xpad[kc][:, dy + 16 * n:
                                                     dy + 16 * n + 16,
                                                     dx:dx + 32]),
                                    start=(idx == 0), stop=(idx == 17))
                                idx += 1
                        nc.scalar.activation(
                            y_sb[mo][:, n * 512:(n + 1) * 512], ps,
                            AF.Identity, bias=cb_sb[:, mo:mo + 1])

        # ---- projections: qT/kT [feat, hw]; v in [hw, feat] ----
        qT_sb = [work.tile([P, HW], F32R, tag=f"qT{g}", name=f"qT{g}") for g in range(2)]
        kT_sb = [work.tile([P, HW], F32R, tag=f"kT{g}", name=f"kT{g}") for g in range(2)]
        with tc.tile_pool(name="ps_proj", bufs=2, space="PSUM") as ps_proj:
            for w_sb, b_sb, src, dst in ((wq_sb, bq_sb, xn_sb, qT_sb),
                                         (wk_sb, bk_sb, yk_sb, kT_sb)):
                for mo in range(2):
                    for n in range(2):
                        ps = ps_proj.tile([P, 512], F32, tag="proj")
                        for kc in range(2):
                            nc.tensor.matmul(
                                ps,
                                lhsT=_mm(w_sb[kc][:, mo * P:(mo + 1) * P]),
                                rhs=_mm(src[kc][:, n * 512:(n + 1) * 512]),
                                start=(kc == 0), stop=(kc == 1))
                        nc.scalar.activation(
                            dst[mo][:, n * 512:(n + 1) * 512], ps,
                            AF.Identity, bias=b_sb[:, mo:mo + 1])

            # v_sb[:, j, h, 0:32] = v rows, [..., 32] = 1.0 (softmax denom)
            v_sb = work.tile([P, 8, HEADS, DV + 1], F32R, tag="vsb")
            nc.gpsimd.memset(v_sb.bitcast(F32), 1.0)
            for j in range(8):
                ps = ps_proj.tile([P, C], F32, tag="vproj")
                for ko in range(2):
                    nc.tensor.matmul(
                        ps,
                        lhsT=_mm(yv_sb[ko][:, j * P:(j + 1) * P]),
                        rhs=_mm(wv_sb[ko]),
                        start=(ko == 0), stop=(ko == 1))
                nc.vector.tensor_add(
                    v_sb[:, j, :, 0:DV],
                    ps.rearrange("p (h d) -> p h d", h=HEADS),
                    bv_bc.rearrange("p (h d) -> p h d", h=HEADS))

        # ---- attention, S^T orientation ----
        scale = float(DK) ** -0.5
        oT_sb = [work.tile([P, HW], F32R, tag=f"oT{g}", name=f"oT{g}") for g in range(2)]
        with tc.tile_pool(name="ps_s", bufs=4, space="PSUM") as ps_s, \
                tc.tile_pool(name="ps_o", bufs=1, space="PSUM") as ps_o:
            for hg in range(2):
                for n in range(2):
                    nsl = slice(n * 512, (n + 1) * 512)
                    po = [ps_o.tile([DV + 1, 512], F32, tag=f"po{hi}", name=f"po{hi}")
                          for hi in range(4)]
                    for j in range(8):
                        for hi in range(4):
                            h = hg * 4 + hi
                            bt_t = bt_pool.tile([P, 512], F32, tag="bt")
                            nc.sync.dma_start(
                                out=bt_t,
                                in_=bt[h, j * P:(j + 1) * P, nsl])
                            s_ps = ps_s.tile([P, 512], F32, tag="s")
                            nc.tensor.matmul(
                                s_ps,
                                lhsT=_mm(kT_sb[hg][32 * hi:32 * (hi + 1),
                                                   j * P:(j + 1) * P]),
                                rhs=_mm(qT_sb[hg][32 * hi:32 * (hi + 1), nsl]),
                                start=True, stop=True,
                                tile_position=(32 * hi, 0))
                            pt = pt_pool.tile([P, 512], F32R, tag="pt")
                            nc.vector.scalar_tensor_tensor(
                                pt, s_ps, scale, bt_t,
                                op0=ALU.mult, op1=ALU.add)
                            nc.scalar.activation(pt, pt, AF.Exp)
                            nc.tensor.matmul(
                                po[hi],
                                lhsT=_mm(v_sb[:, j, h, :]),
                                rhs=_mm(pt),
                                start=(j == 0), stop=(j == 7))
                    for hi in range(4):
                        rec = small.tile([1, 512], F32, tag="rec")
                        nc.vector.reciprocal(rec, po[hi][DV:DV + 1, :])
                        rec_bc = small.tile([DV, 512], F32, tag="recbc")
                        nc.gpsimd.partition_broadcast(rec_bc, rec)
                        nc.vector.tensor_mul(
                            oT_sb[hg][32 * hi:32 * (hi + 1), nsl],
                            po[hi][0:DV, :], rec_bc)

        # ---- output projection + residual ----
        xres_view = xf[0:C, :].rearrange("a (b c) -> (a b) c", b=4)
        with tc.tile_pool(name="ps_out", bufs=2, space="PSUM") as ps_out:
            for m in range(8):
                ps = ps_out.tile([P, C], F32, tag="out")
                for kg in range(2):
                    nc.tensor.matmul(
                        ps,
                        lhsT=_mm(oT_sb[kg][:, m * P:(m + 1) * P]),
                        rhs=_mm(wo_sb[kg]),
                        start=(kg == 0), stop=(kg == 1))
                o_sb = small.tile([P, C], F32, tag="osb")
                nc.vector.tensor_add(o_sb, ps, bo_bc)
                xr_sb = small.tile([P, C], F32, tag="xres")
                nc.sync.dma_start(out=xr_sb,
                                  in_=xres_view[m * P:(m + 1) * P, :])
                nc.vector.tensor_add(o_sb, o_sb, xr_sb)
                nc.sync.dma_start(out=out[m * P:(m + 1) * P, :], in_=o_sb)

        for p in (small, pt_pool, bt_pool, xs_pool, work, consts):
            p.release()

    nc.compile()
    return nc


_NC_CACHE = None


def _get_nc():
    global _NC_CACHE
    if _NC_CACHE is None:
        _NC_CACHE = build_nc()
    return _NC_CACHE


def make_in_maps(x, Wq, bq, conv_k_w, conv_k_b, Wk, bk, conv_v_w, conv_v_b,
                 Wv, bv, Wo, bo, B):
    f = lambda a: np.ascontiguousarray(np.asarray(a), dtype=np.float32)
    xr = f(x).reshape(8, C, HW)
    common = {
        "wq": f(Wq), "bq": f(bq), "wk": f(Wk), "bk": f(bk),
        "wv": f(Wv), "bv": f(bv), "wo": f(Wo), "bo": f(bo),
        "ck": np.ascontiguousarray(f(conv_k_w).transpose(2, 3, 1, 0))
        .reshape(9, C, C),
        "cbk": f(conv_k_b),
        "cv": np.ascontiguousarray(f(conv_v_w).transpose(2, 3, 1, 0))
        .reshape(9, C, C),
        "cbv": f(conv_v_b),
        "bt": np.ascontiguousarray(f(B)[0].transpose(0, 2, 1)),
    }
    return [dict(common, xf=np.ascontiguousarray(np.roll(xr, -b, axis=0))
                 .reshape(2048, HW)) for b in range(8)]


def run(in_maps, **kwargs):
    nc = _get_nc()
    return run_bass_kernel_spmd(nc, in_maps, core_ids=list(range(N_CORES)),
                                **kwargs)


def kernel(**inputs) -> np.ndarray:
    res = run(make_in_maps(**inputs))
    return np.stack([res.results[b]["out"].reshape(C, 32, 32)
                     for b in range(8)])


# revision 6
# speedup vs baseline: 1.2849x; 1.2849x over previous
"""LMHSA (conv-augmented multi-head self-attention block) on 8 trn2 NeuronCores.

Sharding: data-parallel over batch N=8 -> one batch per core; params + B
replicated. The whole-tensor LayerNorm stats are computed redundantly on every
core from the full x (each core's xf input is the full x rolled so its own
batch is rows 0:256).

Per-core layout convention: features on partitions, spatial (hw=1024) on the
free dim.  Attention is computed in S^T orientation (kv on partitions, q on
free) so softmax-exp output feeds the AV matmul directly with no transposes;
the softmax denominator comes from an appended ones-column in v ("v_aug").
Softmax max-subtraction is skipped (logits are O(1), exp cannot overflow).
"""

import ml_dtypes
import numpy as np

import concourse.bass as bass
import concourse.mybir as mybir
import concourse.tile as tile
from concourse import bacc
from concourse.bass_utils import run_bass_kernel_spmd

N_CORES = 8
C = 256
HW = 1024
HEADS = 8
DK = 32
DV = 32
EPS = 1e-5
P = 128
F32 = mybir.dt.float32
F32R = mybir.dt.float32r
BF16 = mybir.dt.bfloat16
AF = mybir.ActivationFunctionType
ALU = mybir.AluOpType

# matmul input dtype: float32r streams 1 row/cycle (vs 4 for float32) when the
# output free dim is >= 256; storage is identical fp32 bits.
FP32R = True


def _mm(t):
    return t


def build_nc():
    nc = bacc.Bacc("TRN2", target_bir_lowering=False, debug=False,
                   num_devices=N_CORES)

    xf = nc.dram_tensor("xf", [2048, HW], F32, kind="ExternalInput").ap()
    wq = nc.dram_tensor("wq", [C, C], F32R, kind="ExternalInput").ap()
    bq = nc.dram_tensor("bq", [C], F32, kind="ExternalInput").ap()
    wk = nc.dram_tensor("wk", [C, C], F32R, kind="ExternalInput").ap()
    bk = nc.dram_tensor("bk", [C], F32, kind="ExternalInput").ap()
    wv = nc.dram_tensor("wv", [C, C], F32R, kind="ExternalInput").ap()
    bv = nc.dram_tensor("bv", [C], F32, kind="ExternalInput").ap()
    wo = nc.dram_tensor("wo", [C, C], F32R, kind="ExternalInput").ap()
    bo = nc.dram_tensor("bo", [C], F32, kind="ExternalInput").ap()
    ck = nc.dram_tensor("ck", [9, C, C], F32R, kind="ExternalInput").ap()
    cbk = nc.dram_tensor("cbk", [C], F32, kind="ExternalInput").ap()
    cv = nc.dram_tensor("cv", [9, C, C], F32R, kind="ExternalInput").ap()
    cbv = nc.dram_tensor("cbv", [C], F32, kind="ExternalInput").ap()
    bt = nc.dram_tensor("bt", [HEADS, HW, HW], BF16, kind="ExternalInput").ap()
    out = nc.dram_tensor("out", [HW, C], F32, kind="ExternalOutput").ap()

    def bcast_pp(ap, n_part):
        # partition-broadcast view of a 1-D dram AP
        return bass.AP(tensor=ap.tensor, offset=ap.offset,
                       ap=[[0, n_part]] + [list(a) for a in ap.ap])

    with tile.TileContext(nc) as tc:
        consts = tc.alloc_tile_pool(name="consts", bufs=1)
        work = tc.alloc_tile_pool(name="work", bufs=1)
        xs_pool = tc.alloc_tile_pool(name="xs", bufs=3)
        bt_pool = tc.alloc_tile_pool(name="btp", bufs=4)
        pt_pool = tc.alloc_tile_pool(name="ptp", bufs=3)
        pt2_pool = tc.alloc_tile_pool(name="ptp2", bufs=3)
        small = tc.alloc_tile_pool(name="small", bufs=2)

        # ---- constants / weights to SBUF ----
        ck_sb, cv_sb, wq_sb, wk_sb, wv_sb, wo_sb = [], [], [], [], [], []
        for kc in range(2):
            t = consts.tile([P, 9, C], F32R, tag=f"ck{kc}")
            nc.sync.dma_start(out=t, in_=ck[:, kc * P:(kc + 1) * P, :]
                              .rearrange("t k o -> k t o"))
            ck_sb.append(t)
            t = consts.tile([P, 9, C], F32R, tag=f"cv{kc}")
            nc.sync.dma_start(out=t, in_=cv[:, kc * P:(kc + 1) * P, :]
                              .rearrange("t k o -> k t o"))
            cv_sb.append(t)
            for nm, src, lst in (("wq", wq, wq_sb), ("wk", wk, wk_sb),
                                 ("wv", wv, wv_sb), ("wo", wo, wo_sb)):
                t = consts.tile([P, C], F32R, tag=f"{nm}{kc}")
                nc.sync.dma_start(out=t, in_=src[kc * P:(kc + 1) * P, :])
                lst.append(t)

        bq_sb = consts.tile([P, 2], F32, tag="bq")
        nc.sync.dma_start(out=bq_sb, in_=bq.rearrange("(m k) -> k m", k=P))
        bk_sb = consts.tile([P, 2], F32, tag="bk")
        nc.sync.dma_start(out=bk_sb, in_=bk.rearrange("(m k) -> k m", k=P))
        cbk_sb = consts.tile([P, 2], F32, tag="cbk")
        nc.sync.dma_start(out=cbk_sb, in_=cbk.rearrange("(m k) -> k m", k=P))
        cbv_sb = consts.tile([P, 2], F32, tag="cbv")
        nc.sync.dma_start(out=cbv_sb, in_=cbv.rearrange("(m k) -> k m", k=P))
        bv_bc = consts.tile([P, C], F32, tag="bvb")
        nc.sync.dma_start(out=bv_bc, in_=bcast_pp(bv, P))
        bo_bc = consts.tile([P, C], F32, tag="bob")
        nc.sync.dma_start(out=bo_bc, in_=bcast_pp(bo, P))

        eps_t = consts.tile([1, 1], F32, tag="eps")
        nc.vector.memset(eps_t, EPS)
        ones_col = consts.tile([P, 1], F32, tag="ones")
        nc.vector.memset(ones_col, 1.0)

        # ---- padded x for the convs (borders zero) ----
        xpad = []
        for kc in range(2):
            t = work.tile([P, 34, 34], F32R, tag=f"xpad{kc}")
            nc.gpsimd.memset(t.bitcast(F32), 0.0)
            nc.sync.dma_start(
                out=t[:, 1:33, 1:33],
                in_=xf[kc * P:(kc + 1) * P, :].rearrange("c (h w) -> c h w", h=32).bitcast(F32R))
            xpad.append(t)

        # ---- whole-tensor layernorm stats over full x (8 MB) ----
        stats_sb = work.tile([P, 32, 6], F32, tag="stats")
        for t_i in range(16):
            xs = xs_pool.tile([P, HW], F32, tag="xs")
            nc.sync.dma_start(out=xs, in_=xf[t_i * P:(t_i + 1) * P, :])
            for c_i in range(2):
                nc.vector.bn_stats(out=stats_sb[:, t_i * 2 + c_i, :],
                                   in_=xs[:, c_i * 512:(c_i + 1) * 512])
        mv = work.tile([P, 2], F32, tag="mv")
        nc.vector.bn_aggr(out=mv, in_=stats_sb)
        s12 = work.tile([P, 2], F32, tag="s12")
        nc.vector.tensor_mul(s12[:, 1:2], mv[:, 0:1], mv[:, 0:1])
        nc.vector.tensor_add(s12[:, 1:2], s12[:, 1:2], mv[:, 1:2])
        nc.vector.tensor_copy(s12[:, 0:1], mv[:, 0:1])

        with tc.tile_pool(name="ps_st", bufs=1, space="PSUM") as ps_st_pool:
            ps_st = ps_st_pool.tile([1, 2], F32, tag="ps_st")
            nc.tensor.matmul(ps_st, lhsT=ones_col, rhs=s12, start=True,
                             stop=True)
            sg = work.tile([1, 2], F32, tag="sg")
            nc.scalar.mul(sg, ps_st, 1.0 / P)
        var_t = work.tile([1, 1], F32, tag="var")
        nc.vector.tensor_mul(var_t, sg[:, 0:1], sg[:, 0:1])
        nc.vector.tensor_sub(var_t, sg[:, 1:2], var_t)
        sd_t = work.tile([1, 1], F32, tag="sd")
        nc.scalar.activation(sd_t, var_t, AF.Sqrt, bias=eps_t)
        s_t = work.tile([1, 1], F32, tag="st")
        nc.vector.reciprocal(s_t, sd_t)
        t_t = work.tile([1, 1], F32, tag="tt")
        nc.vector.tensor_mul(t_t, sg[:, 0:1], s_t)
        nc.scalar.mul(t_t, t_t, -1.0)
        st_small = work.tile([1, 2], F32, tag="stsm")
        nc.vector.tensor_copy(st_small[:, 0:1], s_t)
        nc.vector.tensor_copy(st_small[:, 1:2], t_t)
        st_bc = work.tile([P, 2], F32, tag="stbc")
        nc.gpsimd.partition_broadcast(st_bc, st_small)
        s_bc, t_bc = st_bc[:, 0:1], st_bc[:, 1:2]

        # ---- xn = (x - m) * rsqrt(var + eps)  (only q uses xn) ----
        xn_sb = []
        for kc in range(2):
            t = work.tile([P, HW], F32R, tag=f"xn{kc}")
            nc.vector.tensor_scalar(t, xpad[kc][:, 1:33, 1:33], s_bc, t_bc,
                                    op0=ALU.mult, op1=ALU.add)
            xn_sb.append(t)

        # ---- convs: yk = conv3x3(x, ck) + cbk ; yv likewise ----
        yk_sb = [work.tile([P, HW], F32R, tag=f"yk{mo}", name=f"yk{mo}") for mo in range(2)]
        yv_sb = [work.tile([P, HW], F32R, tag=f"yv{mo}", name=f"yv{mo}") for mo in range(2)]
        with tc.tile_pool(name="ps_conv", bufs=2, space="PSUM") as ps_conv:
            for cw_sb, cb_sb, y_sb in ((ck_sb, cbk_sb, yk_sb),
                                       (cv_sb, cbv_sb, yv_sb)):
                for mo in range(2):
                    for n in range(2):
                        ps = ps_conv.tile([P, 512], F32, tag="conv")
                        idx = 0
                        for tap in range(9):
                            dy, dx = tap // 3, tap % 3
                            for kc in range(2):
                                nc.tensor.matmul(
                                    ps,
                                    lhsT=_mm(cw_sb[kc][:, tap,
                                                       mo * P:(mo + 1) * P]),
                                    rhs=_mm(xpad[kc][:, dy + 16 * n:
                                                     dy + 16 * n + 16,
                                                     dx:dx + 32]),
                                    start=(idx == 0), stop=(idx == 17))
                                idx += 1
                        nc.scalar.activation(
                            y_sb[mo][:, n * 512:(n + 1) * 512], ps,
                            AF.Identity, bias=cb_sb[:, mo:mo + 1])

        # ---- projections: qT/kT [feat, hw] (bf16); v in [hw, feat] ----
        qT_sb = [work.tile([P, HW], BF16, tag=f"qT{g}", name=f"qT{g}") for g in range(2)]
        kT_sb = [work.tile([P, HW], BF16, tag=f"kT{g}", name=f"kT{g}") for g in range(2)]
        with tc.tile_pool(name="ps_proj", bufs=2, space="PSUM") as ps_proj:
            for w_sb, b_sb, src, dst in ((wq_sb, bq_sb, xn_sb, qT_sb),
                                         (wk_sb, bk_sb, yk_sb, kT_sb)):
                for mo in range(2):
                    for n in range(2):
                        ps = ps_proj.tile([P, 512], F32, tag="proj")
                        for kc in range(2):
                            nc.tensor.matmul(
                                ps,
                                lhsT=_mm(w_sb[kc][:, mo * P:(mo + 1) * P]),
                                rhs=_mm(src[kc][:, n * 512:(n + 1) * 512]),
                                start=(kc == 0), stop=(kc == 1))
                        nc.scalar.activation(
                            dst[mo][:, n * 512:(n + 1) * 512], ps,
                            AF.Identity, bias=b_sb[:, mo:mo + 1])

            # v_sb[:, j, h, 0:32] = v rows, [..., 32] = 1.0 (softmax denom)
            v_sb = work.tile([P, 8, HEADS, DV + 1], BF16, tag="vsb")
            nc.gpsimd.memset(v_sb, 1.0)
            for j in range(8):
                ps = ps_proj.tile([P, C], F32, tag="vproj")
                for ko in range(2):
                    nc.tensor.matmul(
                        ps,
                        lhsT=_mm(yv_sb[ko][:, j * P:(j + 1) * P]),
                        rhs=_mm(wv_sb[ko]),
                        start=(ko == 0), stop=(ko == 1))
                nc.vector.tensor_add(
                    v_sb[:, j, :, 0:DV],
                    ps.rearrange("p (h d) -> p h d", h=HEADS),
                    bv_bc.rearrange("p (h d) -> p h d", h=HEADS))

        # ---- attention, S^T orientation; head-pair groups, stage-major ----
        scale = float(DK) ** -0.5
        oT_sb = [work.tile([P, HW], F32R, tag=f"oT{g}", name=f"oT{g}") for g in range(2)]
        with tc.tile_pool(name="ps_s", bufs=2, space="PSUM") as ps_s, \
                tc.tile_pool(name="ps_o", bufs=1, space="PSUM") as ps_o:
            for hg in range(4):
                g, pb = hg // 2, 64 * (hg % 2)
                po = [ps_o.tile([DV + 1, HW], F32, tag=f"po{hi}", name=f"po{hi}")
                      for hi in range(2)]
                for j in range(8):
                    bt_t, s_ps, pt = [], [], []
                    for hi in range(2):
                        h = hg * 2 + hi
                        t = bt_pool.tile([P, HW], BF16, tag="bt", name="bt")
                        nc.sync.dma_start(out=t, in_=bt[h, j * P:(j + 1) * P, :])
                        bt_t.append(t)
                    for hi in range(2):
                        sp = ps_s.tile([P, HW], F32, tag="s", name="s")
                        for n in range(2):
                            nsl = slice(n * 512, (n + 1) * 512)
                            nc.tensor.matmul(
                                sp[:, nsl],
                                lhsT=kT_sb[g][pb + 32 * hi:pb + 32 * (hi + 1),
                                              j * P:(j + 1) * P],
                                rhs=qT_sb[g][pb + 32 * hi:pb + 32 * (hi + 1),
                                             nsl],
                                start=True, stop=True,
                                tile_position=(pb + 32 * hi, 0))
                        s_ps.append(sp)
                    for hi in range(2):
                        t = pt_pool.tile([P, HW], F32, tag="pt", name="pt")
                        nc.vector.scalar_tensor_tensor(
                            t, s_ps[hi], scale, bt_t[hi],
                            op0=ALU.mult, op1=ALU.add)
                        pt.append(t)
                    pt2 = []
                    for hi in range(2):
                        t = pt2_pool.tile([P, HW], BF16, tag="pt2", name="pt2")
                        nc.scalar.activation(t, pt[hi], AF.Exp)
                        pt2.append(t)
                    for hi in range(2):
                        h = hg * 2 + hi
                        for n in range(2):
                            nsl = slice(n * 512, (n + 1) * 512)
                            nc.tensor.matmul(
                                po[hi][:, nsl],
                                lhsT=v_sb[:, j, h, :],
                                rhs=pt2[hi][:, nsl],
                                start=(j == 0), stop=(j == 7))
                for hi in range(2):
                    rec = small.tile([1, HW], F32, tag="rec")
                    nc.vector.reciprocal_approx_fast(rec, po[hi][DV:DV + 1, :])
                    rec_bc = small.tile([DV, HW], F32, tag="recbc")
                    nc.gpsimd.partition_broadcast(rec_bc, rec)
                    nc.vector.tensor_mul(
                        oT_sb[g][pb + 32 * hi:pb + 32 * (hi + 1), :],
                        po[hi][0:DV, :], rec_bc)

        # ---- output projection + residual ----
        xres_view = xf[0:C, :].rearrange("a (b c) -> (a b) c", b=4)
        with tc.tile_pool(name="ps_out", bufs=2, space="PSUM") as ps_out:
            for m in range(8):
                ps = ps_out.tile([P, C], F32, tag="out")
                for kg in range(2):
                    nc.tensor.matmul(
                        ps,
                        lhsT=_mm(oT_sb[kg][:, m * P:(m + 1) * P]),
                        rhs=_mm(wo_sb[kg]),
                        start=(kg == 0), stop=(kg == 1))
                o_sb = small.tile([P, C], F32, tag="osb")
                nc.vector.tensor_add(o_sb, ps, bo_bc)
                xr_sb = small.tile([P, C], F32, tag="xres")
                nc.sync.dma_start(out=xr_sb,
                                  in_=xres_view[m * P:(m + 1) * P, :])
                nc.vector.tensor_add(o_sb, o_sb, xr_sb)
                nc.sync.dma_start(out=out[m * P:(m + 1) * P, :], in_=o_sb)

        for p in (small, pt2_pool, pt_pool, bt_pool, xs_pool, work, consts):
            p.release()

    nc.compile()
    return nc


_NC_CACHE = None


def _get_nc():
    global _NC_CACHE
    if _NC_CACHE is None:
        _NC_CACHE = build_nc()
    return _NC_CACHE


def make_in_maps(x, Wq, bq, conv_k_w, conv_k_b, Wk, bk, conv_v_w, conv_v_b,
                 Wv, bv, Wo, bo, B):
    f = lambda a: np.ascontiguousarray(np.asarray(a), dtype=np.float32)
    xr = f(x).reshape(8, C, HW)
    common = {
        "wq": f(Wq), "bq": f(bq), "wk": f(Wk), "bk": f(bk),
        "wv": f(Wv), "bv": f(bv), "wo": f(Wo), "bo": f(bo),
        "ck": np.ascontiguousarray(f(conv_k_w).transpose(2, 3, 1, 0))
        .reshape(9, C, C),
        "cbk": f(conv_k_b),
        "cv": np.ascontiguousarray(f(conv_v_w).transpose(2, 3, 1, 0))
        .reshape(9, C, C),
        "cbv": f(conv_v_b),
        "bt": np.ascontiguousarray(f(B)[0].transpose(0, 2, 1))
        .astype(ml_dtypes.bfloat16),
    }
    return [dict(common, xf=np.ascontiguousarray(np.roll(xr, -b, axis=0))
                 .reshape(2048, HW)) for b in range(8)]


def run(in_maps, **kwargs):
    nc = _get_nc()
    return run_bass_kernel_spmd(nc, in_maps, core_ids=list(range(N_CORES)),
                                **kwargs)


def kernel(**inputs) -> np.ndarray:
    res = run(make_in_maps(**inputs))
    return np.stack([res.results[b]["out"].reshape(C, 32, 32)
                     for b in range(8)])
